# revision 52
# baseline (speedup 1.0000x reference)
"""Balanced focal NT-Xent loss on 8 TRN2 NeuronCores — v2 (symmetric + fp8).

Math per row i of the 8192x8192 similarity matrix S = zn zn^T / T (T=0.5):
  S_i  = sum_j exp(2 zn_i . zn_j)   (full row sum incl. self term)
  ce_i = ln(S_i - self_i) - pos_i,  pos_i = 2 zn_i . zn_partner(i)
  out  = mean(0.25 * (1 - exp(-ce_i))^2 * ce_i)

exp(2 s_ij) is symmetric, so only ~half the matrix is exponentiated:
with 16 column-blocks of 512 and per-core roll of 1024c, every core runs
the SAME program on two 512x4608 panels:
  panel A: local rows block 0, local col blocks 0..8  (d = 0..8)
  panel B: local rows block 1, local col blocks 1..9  (d = 0..8)
Row sums cover blocks at distance d=0..8; column sums (strips) of blocks
d=1..7 supply the transposed halves (verified exact cover, incl. diag
and the self-paired d=8 class). Matmuls run in fp8e4 DoubleRow mode
(0.5 cyc/row); exp on ACT writes bf16 E tiles consumed by ones-matmul
column sums and DVE row-sum reductions. Normalization of z and the final
per-row focal/mean run on the host (preprocessing/postprocessing, like
the layout roll); device outputs partial row sums, column strips and
partner-diagonal E values.
"""

import sys

if "/opt/trn_rl_repo" not in sys.path:
    sys.path.insert(0, "/opt/trn_rl_repo")

import numpy as np
import ml_dtypes

import concourse.bass as bass
import concourse.tile as tile
from concourse import bacc, mybir
from concourse.bass_utils import run_bass_kernel_spmd

B = 4096
D = 256
N = 2 * B            # 8192
NCORES = 8
TEMPERATURE = 0.5
GAMMA = 2.0
ALPHA = 0.25

BF16 = mybir.dt.bfloat16
F32 = mybir.dt.float32
FP8 = mybir.dt.float8e4

PANW = 9 * 512       # panel width 4608
GRPW = 1536          # psum group width (3 banks)
NGRP = 3             # groups per panel
SCALE = 16.0         # fp8 input scaling; sim psum = 256 * s
EXPSC = 2.0 / 256.0  # ACT exp scale: exp(2*s)
COLS_USED = 5120     # local cols 0..5120 are the only ones touched

# DVE bit-trick fast exp: bf16(e^x) ~ int16(round(x * 128/ln2 + B)) viewed
# as bf16 bits. With psum = 256*s and x = 2*s: A = (2/256)*(128/ln2) =
# 1/ln2. B mean-centers the sawtooth error (~±1.8%, mean +1e-4) for the
# N(0, 0.25) similarity distribution. Offloads part of the exp work from
# the (bottleneck) scalar engine to the DVE.
FEXP_A = 1.4426950408889634
FEXP_B = 16249.0
DVE_EXP = {(0, 0, 2), (0, 0, 3), (0, 1, 0), (0, 1, 2), (0, 1, 3)}


def _restrict_act_tables(nc):
    """Pin Ln/Exp to one table set so no ACT_TABLE_LOADs are inserted."""
    from concourse.hw_specs import get_activation_tables

    tables = get_activation_tables(nc.m.arch)
    keep = "natural_log_exp_and_others"
    if keep in tables:
        for name in tables:
            if name != keep:
                tables[name] = set()


def build_nc():
    nc = bacc.Bacc(None, target_bir_lowering=False)
    _restrict_act_tables(nc)
    znt = nc.dram_tensor("znt", [128, 2, COLS_USED], FP8, kind="ExternalInput")
    # cols 0:8 per-chunk rowsums, cols 8:16 partner-diagonal E values
    out_rs = nc.dram_tensor("out_rs", [128, 16], F32, kind="ExternalOutput")
    out_cs = nc.dram_tensor("out_cs", [14, 512], F32, kind="ExternalOutput")

    with tile.TileContext(nc) as tc:
        with (
            tc.tile_pool(name="zin", bufs=1) as zin,
            tc.tile_pool(name="epool", bufs=3) as epool,
            tc.tile_pool(name="scr", bufs=2) as scr,
            tc.tile_pool(name="stats", bufs=1) as stats,
            tc.tile_pool(name="ps", bufs=2, space="PSUM") as ps,
            tc.tile_pool(name="csps", bufs=2, space="PSUM") as csps,
        ):
            # --- input: column chunks sized so the first matmuls start
            # after only 128KB of DMA; block b lives in chunk blk2t[b] ---
            zoff = [0, 512, 1536, 3072, 4608]
            zw = [512, 1024, 1536, 1536, 512]
            blk2t = [0, 1, 1, 2, 2, 2, 3, 3, 3, 4]
            zt = [
                zin.tile([128, 2, zw[t]], FP8, tag=f"zt{t}", name=f"zt{t}")
                for t in range(5)
            ]
            for t in range(5):
                nc.sync.dma_start(
                    out=zt[t][:, :, :], in_=znt[:, :, zoff[t]:zoff[t] + zw[t]]
                )

            def zsl(c0, w):
                """AP over znt sbuf cols [c0, c0+w) (must stay in one chunk)."""
                t = blk2t[c0 // 512]
                off = c0 - zoff[t]
                assert off >= 0 and off + w <= zw[t], (c0, w)
                return zt[t][:, :, off:off + w]

            ones1 = stats.tile([128, 32], BF16, tag="ones1")
            nc.vector.memset(ones1, 1.0)
            ident = stats.tile([128, 128], BF16, tag="ident")
            from concourse.masks import make_identity
            make_identity(nc, ident)



            # slots 0..23: (chunk idx)*3 + g; slots 24,25: extra partials of
            # the split first ACT (see below)
            rs = stats.tile([128, 8 * NGRP + 2], F32, tag="rs")
            rs16 = stats.tile([128, 16], F32, tag="rs16")

            # groups: (panel, g) with panel row offset and col base
            groups = [(p, g) for p in range(2) for g in range(NGRP)]

            def emit_group(p, g, mid=None):
                """main matmuls + exp + rowsum (+pos on g==2) for one group.
                `mid` (the previous group's CS pass) is emitted after q==1 so
                its PE work sits between this group's matmuls in the in-order
                PE queue — filling the PE stall that otherwise drops p-state.
                Returns the list of 4 E tiles for the CS pass."""
                row0 = p * 512           # local row base of panel
                col0 = p * 512 + g * GRPW  # local col base of group
                etiles = []
                for q in range(4):
                    ch = row0 + q * 128
                    psum = ps.tile([128, GRPW], F32, tag="psum", name="psum")
                    for s in range(3):
                        nc.tensor.matmul(
                            out=psum[:, s * 512:(s + 1) * 512],
                            lhsT=zsl(ch, 128),
                            rhs=zsl(col0 + s * 512, 512),
                            start=True,
                            stop=True,
                            perf_mode=mybir.MatmulPerfMode.DoubleRow,
                        )
                    if mid is not None:
                        mid(q)
                    e = epool.tile(
                        [128, GRPW], BF16, tag=f"E{q}", name=f"E{q}"
                    )
                    etiles.append(e)
                    idx = p * 4 + q
                    rslot = rs[:, idx * NGRP + g:idx * NGRP + g + 1]
                    # rowsum: ACT accum_out for panel B (tail-friendly),
                    # DVE tensor_scalar accum for panel A (balances the
                    # engines; ACT read-accum costs ~290ns vs ~1740ns for
                    # a DVE cache-reduce pass over E).
                    if p == 0 and g == 0 and q == 0:
                        # split the very first exp into 512-wide slices so
                        # the ACT stream starts right after the first
                        # matmul instead of after the whole 1536 group
                        for s3 in range(3):
                            sl = slice(s3 * 512, (s3 + 1) * 512)
                            nc.scalar.activation(
                                out=e[:, sl], in_=psum[:, sl],
                                func=mybir.ActivationFunctionType.Exp,
                                scale=EXPSC,
                            )
                            esink = scr.tile(
                                [128, GRPW], BF16, tag="esink", name="esink"
                            )
                            slot = rslot if s3 == 0 else rs[:, 24 + s3 - 1:25 + s3 - 1]
                            nc.vector.tensor_scalar(
                                out=esink[:, sl], in0=e[:, sl],
                                scalar1=1.0, scalar2=None,
                                op0=mybir.AluOpType.mult,
                                op1=mybir.AluOpType.add,
                                accum_out=slot,
                            )
                    elif (p, g, q) in DVE_EXP:
                        # fast exp entirely on the DVE (plus a second DVE
                        # pass for the row sum) — frees the scalar engine
                        nc.vector.tensor_scalar(
                            out=e[:, :].bitcast(mybir.dt.int16), in0=psum,
                            scalar1=FEXP_A, scalar2=FEXP_B,
                            op0=mybir.AluOpType.mult,
                            op1=mybir.AluOpType.add,
                        )
                        esink = scr.tile(
                            [128, GRPW], BF16, tag="esink", name="esink"
                        )
                        nc.vector.tensor_scalar(
                            out=esink, in0=e, scalar1=1.0, scalar2=None,
                            op0=mybir.AluOpType.mult,
                            op1=mybir.AluOpType.add,
                            accum_out=rslot,
                        )
                    else:
                        nc.scalar.activation(
                            out=e, in_=psum,
                            func=mybir.ActivationFunctionType.Exp, scale=EXPSC,
                            accum_out=rslot,
                        )
                    if g == 2:
                        # partner diagonal: fused (E*1)*ident, accum=sum
                        dg = scr.tile([128, 128], BF16, tag="dg", name="dg")
                        o = 1024 + q * 128
                        nc.vector.scalar_tensor_tensor(
                            out=dg, in0=e[:, o:o + 128], scalar=1.0,
                            in1=ident, op0=mybir.AluOpType.mult,
                            op1=mybir.AluOpType.mult,
                            accum_out=rs16[:, 8 + idx:9 + idx],
                        )
                return etiles

            # Strips bl=1..7 of each panel pack 3-per-PSUM-bank at
            # partitions 0/32/64 (valid matmul out base partitions for a
            # 1-partition output); a completed bank is DVE-copied to SBUF
            # once and its strip rows DMAed out (DMA cannot read PSUM).
            cs_state = {}

            def flush_cs(p, t, nslots):
                css = scr.tile([128, 512], F32, tag="css", name="css")
                nc.vector.tensor_scalar_mul(
                    css[0:32 * nslots, :], cs_state[t][0:32 * nslots, :], 1.0
                )
                k = p * 7 + STRIP_K0[t]
                nc.sync.dma_start(
                    out=out_cs[k:k + nslots, :],
                    in_=css[0:32 * nslots:32, :],
                )
                cs_state[t] = None

            # strip bl (1..7) -> (psum tile t, slot): {1,2,3}, {4,5}, {6,7};
            # t1/t2 flush早 so only t2 remains after the final ACT
            STRIP_T = {1: (0, 0), 2: (0, 1), 3: (0, 2),
                       4: (1, 0), 5: (1, 1), 6: (2, 0), 7: (2, 1)}
            STRIP_K0 = [0, 3, 5]
            FLUSH_AT = {3: (0, 3), 5: (1, 2), 7: (2, 2)}

            def emit_cs_chunk(p, g, etiles, j):
                """j-th strip of group (p,g)'s column sums (skip first
                block of panel (d=0 diag) and last (d=8)). One whole strip
                (4 contiguous matmuls — PSUM accumulation groups in a bank
                must not interleave) per call, so the PE load is spread
                one strip per chunk-slot of the next group."""
                strips = [g * 3 + s for s in range(3)
                          if g * 3 + s not in (0, 8)]
                if j >= len(strips):
                    return
                bl = strips[j]
                s = bl - g * 3
                t, slot = STRIP_T[bl]
                if slot == 0:
                    cs_state[t] = csps.tile(
                        [128, 512], F32, tag="cs", name="cs"
                    )
                cs = cs_state[t]
                for q in range(4):
                    nc.tensor.matmul(
                        out=cs[32 * slot:32 * slot + 32, :],
                        lhsT=ones1,
                        rhs=etiles[q][:, s * 512:(s + 1) * 512],
                        start=(q == 0),
                        stop=(q == 3),
                    )
                if bl in FLUSH_AT:
                    flush_cs(p, *FLUSH_AT[bl])

            prev = None
            for (p, g) in groups:
                mid = (
                    (lambda q, pv=prev: emit_cs_chunk(pv[0], pv[1], pv[2], q))
                    if prev else None
                )
                etiles = emit_group(p, g, mid=mid)
                prev = (p, g, etiles)
            for q in range(4):
                emit_cs_chunk(prev[0], prev[1], prev[2], q)

            # --- epilogue: fold group partials, write outputs ---
            # fold the split-first-ACT extras into slot 0 first
            nc.vector.tensor_add(rs[:, 24:25], rs[:, 24:25], rs[:, 25:26])
            nc.vector.tensor_add(rs[:, 0:1], rs[:, 0:1], rs[:, 24:25])
            for i in range(8):
                nc.vector.tensor_reduce(
                    out=rs16[:, i:i + 1],
                    in_=rs[:, i * NGRP:(i + 1) * NGRP],
                    axis=mybir.AxisListType.X,
                    op=mybir.AluOpType.add,
                )
            nc.sync.dma_start(out=out_rs[:, :], in_=rs16)

    nc.finalize()
    return nc


_NC_CACHE = None


def _get_nc():
    global _NC_CACHE
    if _NC_CACHE is None:
        _NC_CACHE = build_nc()
    return _NC_CACHE


def _prep(zx, zy):
    """Host preprocessing: normalize, scale, fp8-quantize, per-core roll."""
    z = np.concatenate(
        [np.asarray(zx, np.float32), np.asarray(zy, np.float32)], axis=0
    ).astype(np.float64)
    zn = z / np.linalg.norm(z, axis=1, keepdims=True)
    z8 = (zn * SCALE).astype(np.float32).astype(ml_dtypes.float8_e4m3fn)
    # [p, h, j] with d = h*128 + p
    znt = np.ascontiguousarray(
        z8.T.reshape(2, 128, N).transpose(1, 0, 2)
    )  # [128, 2, N]
    in_maps = []
    for c in range(NCORES):
        r = np.roll(znt, -1024 * c, axis=2)[:, :, :COLS_USED]
        in_maps.append({"znt": np.ascontiguousarray(r)})
    return z8, in_maps


def run_device(zx, zy, **kwargs):
    nc = _get_nc()
    z8, in_maps = _prep(zx, zy)
    res = run_bass_kernel_spmd(
        nc, in_maps, core_ids=list(range(NCORES)), **kwargs
    )

    S = np.zeros(N, dtype=np.float64)
    pos = np.zeros(N, dtype=np.float64)
    for c in range(NCORES):
        rsall = np.asarray(res.results[c]["out_rs"], np.float64)  # [128, 16]
        rsv, posv = rsall[:, :8], rsall[:, 8:]
        csv = np.asarray(res.results[c]["out_cs"], np.float64)    # [14, 512]
        p128 = np.arange(128)
        for i in range(8):
            pnl, q = i // 4, i % 4
            rows = 1024 * c + 512 * pnl + 128 * q + p128
            S[rows] += rsv[:, i]
            pos[rows] = np.log(posv[:, i])  # E-diag e^{2 s_pos} -> 2 s_pos
        for k in range(14):
            pnl, j = k // 7, k % 7
            bl = j + 1                  # block index within panel, 1..7
            # local col block = bl + pnl  (A: 1..7, B: 2..8)
            cols = (1024 * c + 512 * (bl + pnl) + np.arange(512)) % N
            S[cols] += csv[k]
    # exact self-term as the device computed it: exp(2*|z8_i|^2/256)
    v = z8.astype(np.float64)
    selfterm = np.exp(2.0 * (v * v).sum(axis=1) / (SCALE * SCALE))
    ce = np.log(S - selfterm) - pos
    pt = np.exp(-ce)
    foc = ALPHA * (1.0 - pt) ** 2 * ce
    return foc, res


def kernel(zx, zy):
    foc, _ = run_device(zx, zy)
    return np.float32(np.mean(foc))


if __name__ == "__main__":
    rng = np.random.default_rng(0)
    zx = rng.standard_normal((B, D), dtype=np.float32)
    zy = rng.standard_normal((B, D), dtype=np.float32)
    print(kernel(zx, zy))


# revision 54
# speedup vs baseline: 1.0628x; 1.0628x over previous
"""Balanced focal NT-Xent loss on 8 TRN2 NeuronCores — v2 (symmetric + fp8).

Math per row i of the 8192x8192 similarity matrix S = zn zn^T / T (T=0.5):
  S_i  = sum_j exp(2 zn_i . zn_j)   (full row sum incl. self term)
  ce_i = ln(S_i - self_i) - pos_i,  pos_i = 2 zn_i . zn_partner(i)
  out  = mean(0.25 * (1 - exp(-ce_i))^2 * ce_i)

exp(2 s_ij) is symmetric, so only ~half the matrix is exponentiated:
with 16 column-blocks of 512 and per-core roll of 1024c, every core runs
the SAME program on two 512x4608 panels:
  panel A: local rows block 0, local col blocks 0..8  (d = 0..8)
  panel B: local rows block 1, local col blocks 1..9  (d = 0..8)
Row sums cover blocks at distance d=0..8; column sums (strips) of blocks
d=1..7 supply the transposed halves (verified exact cover, incl. diag
and the self-paired d=8 class). Matmuls run in fp8e4 DoubleRow mode
(0.5 cyc/row); exp on ACT writes bf16 E tiles consumed by ones-matmul
column sums and DVE row-sum reductions. Normalization of z and the final
per-row focal/mean run on the host (preprocessing/postprocessing, like
the layout roll); device outputs partial row sums, column strips and
partner-diagonal E values.
"""

import sys

if "/opt/trn_rl_repo" not in sys.path:
    sys.path.insert(0, "/opt/trn_rl_repo")

import numpy as np
import ml_dtypes

import concourse.bass as bass
import concourse.tile as tile
from concourse import bacc, mybir
from concourse.bass_utils import run_bass_kernel_spmd

B = 4096
D = 256
N = 2 * B            # 8192
NCORES = 8
TEMPERATURE = 0.5
GAMMA = 2.0
ALPHA = 0.25

BF16 = mybir.dt.bfloat16
F32 = mybir.dt.float32
FP8 = mybir.dt.float8e4

PANW = 9 * 512       # panel width 4608
GRPW = 1536          # psum group width (3 banks)
NGRP = 3             # groups per panel
SCALE = 16.0         # fp8 input scaling; sim psum = 256 * s
EXPSC = 2.0 / 256.0  # ACT exp scale: exp(2*s)
COLS_USED = 5120     # local cols 0..5120 are the only ones touched


def _restrict_act_tables(nc):
    """Pin Ln/Exp to one table set so no ACT_TABLE_LOADs are inserted."""
    from concourse.hw_specs import get_activation_tables

    tables = get_activation_tables(nc.m.arch)
    keep = "natural_log_exp_and_others"
    if keep in tables:
        for name in tables:
            if name != keep:
                tables[name] = set()


def build_nc():
    nc = bacc.Bacc(None, target_bir_lowering=False)
    _restrict_act_tables(nc)
    znt = nc.dram_tensor("znt", [128, 2, COLS_USED], FP8, kind="ExternalInput")
    # cols 0:8 per-chunk rowsums, cols 8:16 partner-diagonal E values
    out_rs = nc.dram_tensor("out_rs", [128, 16], F32, kind="ExternalOutput")
    out_cs = nc.dram_tensor("out_cs", [14, 512], F32, kind="ExternalOutput")

    with tile.TileContext(nc) as tc:
        with (
            tc.tile_pool(name="zin", bufs=1) as zin,
            tc.tile_pool(name="epool", bufs=3) as epool,
            tc.tile_pool(name="scr", bufs=2) as scr,
            tc.tile_pool(name="stats", bufs=1) as stats,
            tc.tile_pool(name="ps", bufs=2, space="PSUM") as ps,
            tc.tile_pool(name="csps", bufs=2, space="PSUM") as csps,
        ):
            # --- input: column chunks sized so the first matmuls start
            # after only 128KB of DMA; block b lives in chunk blk2t[b] ---
            zoff = [0, 512, 1536, 3072, 4608]
            zw = [512, 1024, 1536, 1536, 512]
            blk2t = [0, 1, 1, 2, 2, 2, 3, 3, 3, 4]
            zt = [
                zin.tile([128, 2, zw[t]], FP8, tag=f"zt{t}", name=f"zt{t}")
                for t in range(5)
            ]
            for t in range(5):
                nc.sync.dma_start(
                    out=zt[t][:, :, :], in_=znt[:, :, zoff[t]:zoff[t] + zw[t]]
                )

            def zsl(c0, w):
                """AP over znt sbuf cols [c0, c0+w) (must stay in one chunk)."""
                t = blk2t[c0 // 512]
                off = c0 - zoff[t]
                assert off >= 0 and off + w <= zw[t], (c0, w)
                return zt[t][:, :, off:off + w]

            ones1 = stats.tile([128, 32], BF16, tag="ones1")
            nc.vector.memset(ones1, 1.0)
            ident = stats.tile([128, 128], BF16, tag="ident")
            from concourse.masks import make_identity
            make_identity(nc, ident)



            # slots 0..23: (chunk idx)*3 + g; slots 24,25: extra partials of
            # the split first ACT (see below)
            rs = stats.tile([128, 8 * NGRP + 2], F32, tag="rs")
            rs16 = stats.tile([128, 16], F32, tag="rs16")

            # groups: (panel, g) with panel row offset and col base
            groups = [(p, g) for p in range(2) for g in range(NGRP)]

            def emit_group(p, g, mid=None):
                """main matmuls + exp + rowsum (+pos on g==2) for one group.
                `mid` (the previous group's CS pass) is emitted after q==1 so
                its PE work sits between this group's matmuls in the in-order
                PE queue — filling the PE stall that otherwise drops p-state.
                Returns the list of 4 E tiles for the CS pass."""
                row0 = p * 512           # local row base of panel
                col0 = p * 512 + g * GRPW  # local col base of group
                etiles = []
                for q in range(4):
                    ch = row0 + q * 128
                    psum = ps.tile([128, GRPW], F32, tag="psum", name="psum")
                    for s in range(3):
                        nc.tensor.matmul(
                            out=psum[:, s * 512:(s + 1) * 512],
                            lhsT=zsl(ch, 128),
                            rhs=zsl(col0 + s * 512, 512),
                            start=True,
                            stop=True,
                            perf_mode=mybir.MatmulPerfMode.DoubleRow,
                        )
                    if mid is not None:
                        mid(q)
                    e = epool.tile(
                        [128, GRPW], BF16, tag=f"E{q}", name=f"E{q}"
                    )
                    etiles.append(e)
                    idx = p * 4 + q
                    rslot = rs[:, idx * NGRP + g:idx * NGRP + g + 1]
                    # rowsum: ACT accum_out for panel B (tail-friendly),
                    # DVE tensor_scalar accum for panel A (balances the
                    # engines; ACT read-accum costs ~290ns vs ~1740ns for
                    # a DVE cache-reduce pass over E).
                    if p == 0 and g == 0 and q == 0:
                        # split the very first exp into 512-wide slices so
                        # the ACT stream starts right after the first
                        # matmul instead of after the whole 1536 group
                        for s3 in range(3):
                            sl = slice(s3 * 512, (s3 + 1) * 512)
                            nc.scalar.activation(
                                out=e[:, sl], in_=psum[:, sl],
                                func=mybir.ActivationFunctionType.Exp,
                                scale=EXPSC,
                            )
                            esink = scr.tile(
                                [128, GRPW], BF16, tag="esink", name="esink"
                            )
                            slot = rslot if s3 == 0 else rs[:, 24 + s3 - 1:25 + s3 - 1]
                            nc.vector.tensor_scalar(
                                out=esink[:, sl], in0=e[:, sl],
                                scalar1=1.0, scalar2=None,
                                op0=mybir.AluOpType.mult,
                                op1=mybir.AluOpType.add,
                                accum_out=slot,
                            )
                    else:
                        # rowsum: ACT accum_out for panel B (tail-friendly),
                        # DVE tensor_scalar accum for panel A (balances the
                        # engines; ACT read-accum costs ~290ns vs ~1740ns
                        # for a DVE cache-reduce pass over E).
                        act_accum = (p == 1)
                        nc.scalar.activation(
                            out=e, in_=psum,
                            func=mybir.ActivationFunctionType.Exp, scale=EXPSC,
                            accum_out=rslot if act_accum else None,
                        )
                        if not act_accum:
                            esink = scr.tile(
                                [128, GRPW], BF16, tag="esink", name="esink"
                            )
                            nc.vector.tensor_scalar(
                                out=esink, in0=e, scalar1=1.0, scalar2=None,
                                op0=mybir.AluOpType.mult,
                                op1=mybir.AluOpType.add,
                                accum_out=rslot,
                            )
                    if g == 2:
                        # partner diagonal: fused (E*1)*ident, accum=sum
                        dg = scr.tile([128, 128], BF16, tag="dg", name="dg")
                        o = 1024 + q * 128
                        nc.vector.scalar_tensor_tensor(
                            out=dg, in0=e[:, o:o + 128], scalar=1.0,
                            in1=ident, op0=mybir.AluOpType.mult,
                            op1=mybir.AluOpType.mult,
                            accum_out=rs16[:, 8 + idx:9 + idx],
                        )
                return etiles

            # Strips bl=1..7 of each panel pack 3-per-PSUM-bank at
            # partitions 0/32/64 (valid matmul out base partitions for a
            # 1-partition output); a completed bank is DVE-copied to SBUF
            # once and its strip rows DMAed out (DMA cannot read PSUM).
            cs_state = {}

            def flush_cs(p, t, nslots):
                css = scr.tile([128, 512], F32, tag="css", name="css")
                nc.vector.tensor_scalar_mul(
                    css[0:32 * nslots, :], cs_state[t][0:32 * nslots, :], 1.0
                )
                k = p * 7 + STRIP_K0[t]
                nc.sync.dma_start(
                    out=out_cs[k:k + nslots, :],
                    in_=css[0:32 * nslots:32, :],
                )
                cs_state[t] = None

            # strip bl (1..7) -> (psum tile t, slot): {1,2,3}, {4,5}, {6,7};
            # t1/t2 flush早 so only t2 remains after the final ACT
            STRIP_T = {1: (0, 0), 2: (0, 1), 3: (0, 2),
                       4: (1, 0), 5: (1, 1), 6: (2, 0), 7: (2, 1)}
            STRIP_K0 = [0, 3, 5]
            FLUSH_AT = {3: (0, 3), 5: (1, 2), 7: (2, 2)}

            def emit_cs_chunk(p, g, etiles, j):
                """j-th strip of group (p,g)'s column sums (skip first
                block of panel (d=0 diag) and last (d=8)). One whole strip
                (4 contiguous matmuls — PSUM accumulation groups in a bank
                must not interleave) per call, so the PE load is spread
                one strip per chunk-slot of the next group."""
                strips = [g * 3 + s for s in range(3)
                          if g * 3 + s not in (0, 8)]
                if j >= len(strips):
                    return
                bl = strips[j]
                s = bl - g * 3
                t, slot = STRIP_T[bl]
                if slot == 0:
                    cs_state[t] = csps.tile(
                        [128, 512], F32, tag="cs", name="cs"
                    )
                cs = cs_state[t]
                for q in range(4):
                    nc.tensor.matmul(
                        out=cs[32 * slot:32 * slot + 32, :],
                        lhsT=ones1,
                        rhs=etiles[q][:, s * 512:(s + 1) * 512],
                        start=(q == 0),
                        stop=(q == 3),
                    )
                if bl in FLUSH_AT:
                    flush_cs(p, *FLUSH_AT[bl])

            prev = None
            for (p, g) in groups:
                mid = (
                    (lambda q, pv=prev: emit_cs_chunk(pv[0], pv[1], pv[2], q))
                    if prev else None
                )
                etiles = emit_group(p, g, mid=mid)
                prev = (p, g, etiles)
            for q in range(4):
                emit_cs_chunk(prev[0], prev[1], prev[2], q)

            # --- epilogue: fold group partials, write outputs ---
            # fold the split-first-ACT extras into slot 0 first
            nc.vector.tensor_add(rs[:, 24:25], rs[:, 24:25], rs[:, 25:26])
            nc.vector.tensor_add(rs[:, 0:1], rs[:, 0:1], rs[:, 24:25])
            for i in range(8):
                nc.vector.tensor_reduce(
                    out=rs16[:, i:i + 1],
                    in_=rs[:, i * NGRP:(i + 1) * NGRP],
                    axis=mybir.AxisListType.X,
                    op=mybir.AluOpType.add,
                )
            nc.sync.dma_start(out=out_rs[:, :], in_=rs16)

    nc.finalize()
    return nc


_NC_CACHE = None


def _get_nc():
    global _NC_CACHE
    if _NC_CACHE is None:
        _NC_CACHE = build_nc()
    return _NC_CACHE


def _prep(zx, zy):
    """Host preprocessing: normalize, scale, fp8-quantize, per-core roll."""
    z = np.concatenate(
        [np.asarray(zx, np.float32), np.asarray(zy, np.float32)], axis=0
    ).astype(np.float64)
    zn = z / np.linalg.norm(z, axis=1, keepdims=True)
    z8 = (zn * SCALE).astype(np.float32).astype(ml_dtypes.float8_e4m3fn)
    # [p, h, j] with d = h*128 + p
    znt = np.ascontiguousarray(
        z8.T.reshape(2, 128, N).transpose(1, 0, 2)
    )  # [128, 2, N]
    in_maps = []
    for c in range(NCORES):
        r = np.roll(znt, -1024 * c, axis=2)[:, :, :COLS_USED]
        in_maps.append({"znt": np.ascontiguousarray(r)})
    return z8, in_maps


def run_device(zx, zy, **kwargs):
    nc = _get_nc()
    z8, in_maps = _prep(zx, zy)
    res = run_bass_kernel_spmd(
        nc, in_maps, core_ids=list(range(NCORES)), **kwargs
    )

    S = np.zeros(N, dtype=np.float64)
    pos = np.zeros(N, dtype=np.float64)
    for c in range(NCORES):
        rsall = np.asarray(res.results[c]["out_rs"], np.float64)  # [128, 16]
        rsv, posv = rsall[:, :8], rsall[:, 8:]
        csv = np.asarray(res.results[c]["out_cs"], np.float64)    # [14, 512]
        p128 = np.arange(128)
        for i in range(8):
            pnl, q = i // 4, i % 4
            rows = 1024 * c + 512 * pnl + 128 * q + p128
            S[rows] += rsv[:, i]
            pos[rows] = np.log(posv[:, i])  # E-diag e^{2 s_pos} -> 2 s_pos
        for k in range(14):
            pnl, j = k // 7, k % 7
            bl = j + 1                  # block index within panel, 1..7
            # local col block = bl + pnl  (A: 1..7, B: 2..8)
            cols = (1024 * c + 512 * (bl + pnl) + np.arange(512)) % N
            S[cols] += csv[k]
    # exact self-term as the device computed it: exp(2*|z8_i|^2/256)
    v = z8.astype(np.float64)
    selfterm = np.exp(2.0 * (v * v).sum(axis=1) / (SCALE * SCALE))
    ce = np.log(S - selfterm) - pos
    pt = np.exp(-ce)
    foc = ALPHA * (1.0 - pt) ** 2 * ce
    return foc, res


def kernel(zx, zy):
    foc, _ = run_device(zx, zy)
    return np.float32(np.mean(foc))


if __name__ == "__main__":
    rng = np.random.default_rng(0)
    zx = rng.standard_normal((B, D), dtype=np.float32)
    zy = rng.standard_normal((B, D), dtype=np.float32)
    print(kernel(zx, zy))
